# revision 11
# baseline (speedup 1.0000x reference)
"""Trainium2 Bass kernel for EnhancedSeq2Seq (2-layer LSTM enc/dec + attention + 2-expert top-1 MoE vocab head).

Sharding: batch-parallel recurrent part (64/8 = 8 rows per core),
vocab-parallel MoE head (32000/8 = 4000 per core). Token features are
all-gathered across cores in chunks and the MoE projection is pipelined
behind the decoder recurrence.

v2 design notes (vs the original baseline):
  - The recurrence is latency-bound (per-step serial dependency chain of
    ~20 engine ops).  The local batch (8 rows) is split into G=2 groups of
    4 rows whose chains interleave, halving the effective step latency.
  - ih-precomputes and biases are accumulated into the cell PSUM via an
    identity matmul, removing the z-add from the critical path.
  - Gather payload is bf16 (halves collective bytes); the expert-select
    mask m is computed locally in f32 (sign() margins must not be
    perturbed by bf16 rounding) and shipped as a payload row.
  - MoE staging copies run on Pool/Act engines, out of the DVE queue that
    carries the recurrence; each MoE block does 3 payload DMAs and 1
    output DMA (staged in SBUF) instead of 18.

Scale conventions inside the device program:
  - h state tiles hold H = 2*h ("doubled h") so the sigmoid can be computed
    as a single tanh: sigmoid(x) = 0.5 + 0.5*tanh(x/2).  All weights that
    consume h (or doubled context CTX2 = 2*ctx) are pre-halved on the host.
  - encoutT holds doubled encoder outputs, att_WT is pre-halved.
  - MoE expert blend (top-1, K=1 => gate weight == 1):
      out = xf@W1 + (m*xf)@(W0-W1) + b1 + m*(b0-b1),  m = 1 if expert0 wins.
"""

import os
import sys

sys.path.insert(0, "/opt/trn_rl_repo")

import ml_dtypes
import numpy as np

import concourse.bass as bass
import concourse.mybir as mybir
import concourse.tile as tile
from concourse import bacc
from concourse.bass import IndirectOffsetOnAxis
from concourse.bass_utils import run_bass_kernel_spmd
from concourse.masks import make_identity

V, E, H = 32000, 64, 128
B, S, T = 64, 30, 20
NCORES = 8
BL = B // NCORES        # 8   local batch rows
VS = V // NCORES        # 4000 vocab shard
G4 = 4 * H              # 512
NTE = BL * S            # 240  encoder tokens / core
NTD = BL * T            # 160  decoder tokens / core
NBLK = T // 2           # 10   128-token MoE blocks
TOKB = 2 * B            # 128  tokens per MoE block (all cores)
PAYR = 2 * H + 1        # 257  payload rows per step (H1, CTX2, m)
VTILES = [(i * 512, min((i + 1) * 512, VS)) for i in range((VS + 511) // 512)]

G = 2                   # batch pipeline groups
GBL = BL // G           # 4   rows per group
CHUNKS = [6, 6, 6, 2]   # decoder steps per all-gather (must each be even)
assert sum(CHUNKS) == T

f32 = mybir.dt.float32
f32r = mybir.dt.float32r
bf16 = mybir.dt.bfloat16
i32 = mybir.dt.int32
AF = mybir.ActivationFunctionType
ALU = mybir.AluOpType
AX = mybir.AxisListType

_cache = {}

# debug toggles for bisection
DBG_COLLECTIVE = True
DBG_GATHER = True
DBG_PHASE = 4  # 1=setup/embed/ihpre 2=+encoder 3=+decoder 4=+moe/collective


def _build_program():
    nc = bacc.Bacc("TRN2", target_bir_lowering=False, debug=False, num_devices=NCORES)

    # ---------------- I/O -------------------------------------------------
    din = {}

    def dram_in(name, shape, dtype=f32):
        din[name] = nc.dram_tensor(name, list(shape), dtype, kind="ExternalInput")
        return din[name]

    src_idx = dram_in("src_idx", [2, NTE // 2, 1], i32)
    trg_idx = dram_in("trg_idx", [2, NTD // 2, 1], i32)
    emb = dram_in("emb", [V, E])
    wih0T = dram_in("wih0T", [E, G4])
    whh0T = dram_in("whh0T", [H, G4])
    b0g = dram_in("b0g", [H, 4])
    wih1T = dram_in("wih1T", [H, G4])
    whh1T = dram_in("whh1T", [H, G4])
    b1f = dram_in("b1f", [H, 4 * GBL])
    dwih0xT = dram_in("dwih0xT", [E, G4])
    dwih0cT = dram_in("dwih0cT", [H, G4])
    dwhh0T = dram_in("dwhh0T", [H, G4])
    db0g = dram_in("db0g", [H, 4])
    dwih1T = dram_in("dwih1T", [H, G4])
    dwhh1T = dram_in("dwhh1T", [H, G4])
    db1f = dram_in("db1f", [H, 4 * GBL])
    attWT = dram_in("attWT", [H, H])
    attb = dram_in("attb", [H, 1])
    attv = dram_in("attv", [H, 1], bf16)
    wd12 = dram_in("wd12", [H, 2])
    gdb = dram_in("gdb", [1, 1])
    w1a = dram_in("w1a", [H, VS], bf16)
    w1b = dram_in("w1b", [H, VS], bf16)
    wda = dram_in("wda", [H, VS], bf16)
    wdb = dram_in("wdb", [H, VS], bf16)
    bias2 = dram_in("bias2", [2, VS], bf16)

    out = nc.dram_tensor("out", [NBLK * TOKB, VS], bf16, kind="ExternalOutput")

    with tile.TileContext(nc) as tc:
        with (
            tc.tile_pool(name="wc", bufs=1) as wc,            # constants / persistents
            tc.tile_pool(name="sb", bufs=4) as sb,            # rotating work tiles
            tc.tile_pool(name="sb3", bufs=3) as sb3,          # recurrent state tiles
            tc.tile_pool(name="sbm", bufs=2) as sbm,          # MoE activation tiles
            tc.tile_pool(name="sbo", bufs=2) as sbo,          # MoE output staging
            tc.tile_pool(name="ppc", bufs=2, space="PSUM") as ppc,   # cell psums
            tc.tile_pool(name="ppe", bufs=2, space="PSUM") as ppe,   # attention psums
            tc.tile_pool(name="pps", bufs=2, space="PSUM") as pps,   # small psums
            tc.tile_pool(name="ppo", bufs=2, space="PSUM") as ppo,   # MoE out psums
            tc.tile_pool(name="dr", bufs=2, space="DRAM") as dr,     # collective bufs
        ):
            # ---------------- constant loads ------------------------------
            idt = wc.tile([H, H], f32, tag="idt", name="idt")
            make_identity(nc, idt[:])

            # ---------------- embedding gather + transpose ----------------
            def embed(idx_dram, nchunks, chunk, xT):
                for i in range(nchunks):
                    isb = sb.tile([chunk, 1], i32, tag=f"isb{chunk}")
                    nc.sync.dma_start(out=isb[:], in_=idx_dram[i])
                    gat = sb.tile([chunk, E], f32, tag=f"embg{chunk}")
                    if DBG_GATHER:
                        nc.gpsimd.indirect_dma_start(
                            out=gat[:],
                            out_offset=None,
                            in_=emb[:, :],
                            in_offset=IndirectOffsetOnAxis(ap=isb[:, 0:1], axis=0),
                        )
                    else:
                        nc.sync.dma_start(out=gat[:], in_=emb[0:chunk, :])
                    pst = ppo.tile([TOKB, 512], f32, tag="po")
                    nc.tensor.transpose(
                        pst[0:E, 0:chunk], gat[:], idt[0:chunk, 0:chunk]
                    )
                    nc.scalar.copy(
                        out=xT[:, i * chunk : (i + 1) * chunk], in_=pst[0:E, 0:chunk]
                    )

            _ct_count = [0]

            def const_tile(name, shape, dtype=f32, eng=None):
                t = wc.tile(list(shape), dtype, tag=name, name=name)
                if eng is None:
                    eng = nc.sync if _ct_count[0] % 2 == 0 else nc.scalar
                    _ct_count[0] += 1
                eng.dma_start(out=t[:], in_=din[name][:])
                return t

            # encoder-critical first
            xT = wc.tile([E, NTE], f32, tag="xT", name="xT")
            embed(src_idx, 2, NTE // 2, xT)
            c_wih0T = const_tile("wih0T", [E, G4])
            c_whh0T = const_tile("whh0T", [H, G4])
            c_b0g = const_tile("b0g", [H, 4])
            c_wih1T = const_tile("wih1T", [H, G4])
            c_whh1T = const_tile("whh1T", [H, G4])
            c_b1f = const_tile("b1f", [H, 4 * GBL])

            # persistent activations
            ihpre0 = wc.tile([H, S * 4 * BL], f32, tag="ihpre0", name="ihpre0")
            decihp = wc.tile([H, T * 4 * BL], f32, tag="decihp", name="decihp")
            encoutT = wc.tile([H, NTE], f32, tag="encoutT", name="encoutT")   # (b, s) cols
            encprojT = wc.tile([H, NTE], f32, tag="encprojT", name="encprojT")

            # ---------------- ih precomputes ------------------------------
            # layout: cols = (t, grp, gate, b4) so each (t, grp) slice is a
            # contiguous [H, 16] rhs for the identity-matmul accumulate.
            def ih_pre(dst, nt, wT, rhs, bg):
                dview = dst[:].rearrange(
                    "p (t grp g b) -> p t grp g b", t=nt, grp=G, g=4, b=GBL
                )
                pv = None
                for g in range(4):
                    ps = ppo.tile([TOKB, 512], f32, tag="po")
                    nc.tensor.matmul(
                        ps[:, 0 : nt * BL], lhsT=wT[:, g * H : (g + 1) * H], rhs=rhs[:],
                        start=True, stop=True,
                    )
                    pv = ps[:, 0 : nt * BL].rearrange(
                        "p (t grp b) -> p t grp b", t=nt, grp=G, b=GBL
                    )
                    for grp in range(G):
                        nc.scalar.activation(
                            out=dview[:, :, grp, g, :],
                            in_=pv[:, :, grp, :],
                            func=AF.Identity,
                            bias=bg[:, g : g + 1],
                        )

            ih_pre(ihpre0, S, c_wih0T, xT, c_b0g)

            # ---------------- LSTM cell helper ----------------------------
            def lstm_cell(tag, pre_rhs, mats, c_prev, h_out_ap, h_dtype=f32):
                """pre_rhs: [H, 16] SBUF rhs accumulated via identity matmul
                (ih-precompute or a pre-broadcast bias tile).
                mats: list of (lhsT_full[H,512], rhs_ap) accumulated per gate.
                Returns (new c tile, h tile or None).  Writes H (=2h) into
                h_out_ap."""
                ps = ppc.tile([H, 4 * GBL], f32, tag="pz")
                nc.tensor.matmul(ps[:], lhsT=idt[:], rhs=pre_rhs, start=True,
                                 stop=(not mats))
                for j, (lhsT, rhs) in enumerate(mats):
                    for g in range(4):
                        nc.tensor.matmul(
                            ps[:, g * GBL : (g + 1) * GBL],
                            lhsT=lhsT[:, g * H : (g + 1) * H],
                            rhs=rhs,
                            start=False,
                            stop=(j == len(mats) - 1),
                        )
                # z's g-gate block is pre-doubled on the host, so one
                # tanh(0.5*z) covers sigmoid halves AND the true tanh(g).
                tio = sb.tile([H, 4 * GBL], f32, tag="tio_" + tag)
                nc.scalar.activation(out=tio[:], in_=ps[:], func=AF.Tanh, scale=0.5)
                tg = tio[:, 3 * GBL : 4 * GBL]
                # cS carries 2*c ("doubled c"): avoids a separate 0.5 scale op
                bb = sb.tile([H, GBL], f32, tag="bb_" + tag)
                nc.vector.scalar_tensor_tensor(
                    out=bb[:], in0=tio[:, 0:GBL], scalar=1.0, in1=tg,
                    op0=ALU.add, op1=ALU.mult,
                )
                cS = sb3.tile([H, GBL], f32, tag="c_" + tag)
                if c_prev is None:
                    nc.vector.tensor_copy(out=cS[:], in_=bb[:])
                else:
                    aa = sb.tile([H, GBL], f32, tag="aa_" + tag)
                    nc.vector.scalar_tensor_tensor(
                        out=aa[:], in0=tio[:, GBL : 2 * GBL], scalar=1.0, in1=c_prev,
                        op0=ALU.add, op1=ALU.mult,
                    )
                    nc.vector.scalar_tensor_tensor(
                        out=cS[:], in0=aa[:], scalar=0.5, in1=bb[:],
                        op0=ALU.mult, op1=ALU.add,
                    )
                tch = sb.tile([H, GBL], f32, tag="tc_" + tag)
                nc.scalar.activation(out=tch[:], in_=cS[:], func=AF.Tanh, scale=0.5)
                nc.vector.scalar_tensor_tensor(
                    out=h_out_ap, in0=tio[:, 2 * GBL : 3 * GBL], scalar=1.0, in1=tch[:],
                    op0=ALU.add, op1=ALU.mult,
                )
                return cS

            # ---------------- encoder ------------------------------------
            enc_view = encoutT[:].rearrange("p (b s) -> p b s", b=BL, s=S)
            S_eff = S if DBG_PHASE >= 2 else 0
            ih0v = ihpre0[:].rearrange("p (t k) -> p t k", t=S, k=4 * BL)
            h0 = [None] * G
            c0 = [None] * G
            c1 = [None] * G
            h1_ap = [None] * G
            for t in range(S_eff):
                for g in range(G):
                    mats0 = [] if t == 0 else [(c_whh0T, h0[g][:])]
                    h0n = sb3.tile([H, GBL], f32, tag=f"h0e{g}")
                    c0[g] = lstm_cell(
                        f"e0{g}", ih0v[:, t, g * 4 * GBL : (g + 1) * 4 * GBL],
                        mats0, None if c0[g] is None else c0[g][:], h0n[:],
                    )
                    h0[g] = h0n
                    mats1 = ([(c_whh1T, h1_ap[g])] if t > 0 else []) + [(c_wih1T, h0[g][:])]
                    h1_ap[g] = enc_view[:, g * GBL : (g + 1) * GBL, t]
                    c1[g] = lstm_cell(
                        f"e1{g}", c_b1f[:], mats1,
                        None if c1[g] is None else c1[g][:], h1_ap[g],
                    )

            # decoder weights + trg embed: loaded while the encoder runs
            xdT = wc.tile([E, NTD], f32, tag="xdT", name="xdT")
            embed(trg_idx, 2, NTD // 2, xdT)
            c_dwih0xT = const_tile("dwih0xT", [E, G4])
            c_dwih0cT = const_tile("dwih0cT", [H, G4])
            c_dwhh0T = const_tile("dwhh0T", [H, G4])
            c_db0g = const_tile("db0g", [H, 4])
            c_dwih1T = const_tile("dwih1T", [H, G4])
            c_dwhh1T = const_tile("dwhh1T", [H, G4])
            c_db1f = const_tile("db1f", [H, 4 * GBL])
            c_attWT = const_tile("attWT", [H, H])
            c_attb = const_tile("attb", [H, 1])
            c_attv = const_tile("attv", [H, 1], bf16)
            c_wd12 = const_tile("wd12", [H, 2])
            c_gdb = const_tile("gdb", [1, 1])
            ih_pre(decihp, T, c_dwih0xT, xdT, c_db0g)

            ones_l = wc.tile([1, H], bf16, tag="ones_l", name="ones_l")
            nc.vector.memset(ones_l[:], 1.0)
            b2T = wc.tile([2, TOKB], bf16, tag="b2T", name="b2T")
            nc.sync.dma_start(out=b2T[1:2, :], in_=ones_l[:])

            # big MoE weights: emitted after the encoder so their DMA
            # bandwidth doesn't contend with the setup-critical loads
            c_w1a = const_tile("w1a", [H, VS], bf16, eng=nc.scalar)
            c_w1b = const_tile("w1b", [H, VS], bf16, eng=nc.scalar)
            c_wda = const_tile("wda", [H, VS], bf16, eng=nc.scalar)
            c_wdb = const_tile("wdb", [H, VS], bf16, eng=nc.scalar)
            c_bias2 = const_tile("bias2", [2, VS], bf16, eng=nc.scalar)

            # ---------------- encoder projection --------------------------
            run_dec = DBG_PHASE >= 3 and S_eff == S
            if S_eff == S:
                psP = ppo.tile([TOKB, 512], f32, tag="po")
                nc.tensor.matmul(
                    psP[:, 0:NTE], lhsT=c_attWT[:], rhs=encoutT[:], start=True, stop=True
                )
                nc.scalar.activation(
                    out=encprojT[:], in_=psP[:, 0:NTE], func=AF.Identity,
                    bias=c_attb[:, 0:1]
                )
            epj_view = encprojT[:].rearrange("p (b s) -> p b s", b=BL, s=S)

            # ---------------- MoE block ----------------------------------
            def moe_block(blk, gat, s0):
                xf1 = sbm.tile([H, TOKB], bf16, tag="xf1")
                xf2 = sbm.tile([H, TOKB], bf16, tag="xf2")
                for dst, r0 in ((xf1, 0), (xf2, H)):
                    dv = dst[:].rearrange("p (c s b) -> p c s b", c=NCORES, s=2)
                    for sj in range(2):
                        nc.sync.dma_start(
                            out=dv[:, :, sj, :],
                            in_=gat[:, s0 + sj, r0 : r0 + H, :].rearrange(
                                "c r b -> r c b"
                            ),
                        )
                mv = b2T[0:1, :].rearrange("p (c s b) -> p c s b", c=NCORES, s=2)
                for sj in range(2):
                    nc.scalar.dma_start(
                        out=mv[:, :, sj, :],
                        in_=gat[:, s0 + sj, PAYR - 1 : PAYR, :].rearrange(
                            "c r b -> r c b"
                        ),
                    )
                mB = sbm.tile([H, TOKB], bf16, tag="mB")
                nc.gpsimd.partition_broadcast(mB[:], b2T[0:1, :])
                x01 = sbm.tile([H, TOKB], bf16, tag="x01")
                x02 = sbm.tile([H, TOKB], bf16, tag="x02")
                nc.gpsimd.tensor_mul(out=x01[:], in0=xf1[:], in1=mB[:])
                nc.gpsimd.tensor_mul(out=x02[:], in0=xf2[:], in1=mB[:])
                stage = sbo.tile([TOKB, VS], bf16, tag="stage")
                for nv, (lo, hi) in enumerate(VTILES):
                    w = hi - lo
                    po = ppo.tile([TOKB, 512], f32, tag="po")
                    sl = slice(lo, hi)
                    mms = [
                        (xf1, c_w1a), (xf2, c_w1b), (x01, c_wda), (x02, c_wdb), (b2T, c_bias2),
                    ]
                    for j, (lt, rt) in enumerate(mms):
                        nc.tensor.matmul(
                            po[:, 0:w],
                            lhsT=lt[:],
                            rhs=rt[:, sl],
                            start=(j == 0),
                            stop=(j == len(mms) - 1),
                        )
                    if nv % 2 == 0:
                        nc.scalar.copy(out=stage[:, sl], in_=po[:, 0:w])
                    else:
                        nc.vector.tensor_copy(out=stage[:, sl], in_=po[:, 0:w])
                nc.gpsimd.dma_start(
                    out=out[blk * TOKB : (blk + 1) * TOKB, :], in_=stage[:]
                )

            # ---------------- decoder ------------------------------------
            # decoder state starts from the encoder's final (h, c) per layer
            if run_dec:
                h0d_ap = [h0[g][:] for g in range(G)]
                h1d_ap = [enc_view[:, g * GBL : (g + 1) * GBL, S - 1] for g in range(G)]
                c0d_ap = [c0[g][:] for g in range(G)]
                c1d_ap = [c1[g][:] for g in range(G)]
            dihv = decihp[:].rearrange("p (t k) -> p t k", t=T, k=4 * BL)
            bounce = None
            gat_pend = None    # (gat tile, chunk_len)
            blk_base = 0       # first block index of pending gather
            blk_next = 0       # next pending block to emit
            ci = 0             # chunk index
            coff = 0           # first step of current chunk
            for t in range(T if run_dec else 0):
                tc_off = t - coff
                if tc_off == 0:
                    bounce = dr.tile([CHUNKS[ci], PAYR, BL], bf16, tag=f"bounce{CHUNKS[ci]}")
                if tc_off % 2 == 0:
                    mS = sb.tile([1, 2 * BL], f32, tag="mS")
                    mCv = sb.tile([1, 2 * BL], bf16, tag="mCv")
                moff = (tc_off % 2) * BL
                for g in range(G):
                    gs = slice(g * GBL, (g + 1) * GBL)
                    # ---- attention ----
                    egIn = sb.tile([H, GBL * S], f32, tag=f"egin{g}")
                    nc.vector.scalar_tensor_tensor(
                        out=egIn[:].rearrange("p (b s) -> p b s", b=GBL),
                        in0=h1d_ap[g].unsqueeze(2).to_broadcast([H, GBL, S]),
                        scalar=0.5,
                        in1=epj_view[:, gs, :],
                        op0=ALU.mult,
                        op1=ALU.add,
                    )
                    energy = sb.tile([H, GBL * S], bf16, tag=f"energy{g}")
                    nc.scalar.activation(out=energy[:], in_=egIn[:], func=AF.Tanh)
                    pscg = pps.tile([1, 128], f32, tag="pscg")
                    psS = pscg[0:1, 0 : GBL * S]
                    nc.tensor.matmul(psS, lhsT=c_attv[:, 0:1], rhs=energy[:],
                                     start=True, stop=True)
                    eRow = sb.tile([1, GBL * S], bf16, tag=f"eRow{g}")
                    nc.scalar.activation(out=eRow[:], in_=psS, func=AF.Exp)
                    eB = ppe.tile([H, GBL * S], f32, tag="peb")
                    nc.tensor.matmul(eB[:], lhsT=ones_l[:], rhs=eRow[:],
                                     start=True, stop=True)
                    prod = sb.tile([H, GBL * S], f32, tag=f"prod{g}")
                    nc.vector.tensor_mul(
                        out=prod[:],
                        in0=encoutT[:, g * GBL * S : (g + 1) * GBL * S],
                        in1=eB[:],
                    )
                    ctxU = sb.tile([H, GBL], f32, tag=f"ctxU{g}")
                    nc.vector.reduce_sum(
                        out=ctxU[:],
                        in_=prod[:].rearrange("p (b s) -> p b s", b=GBL),
                        axis=AX.X,
                    )
                    den = sb.tile([H, GBL], f32, tag=f"den{g}")
                    nc.vector.reduce_sum(
                        out=den[:],
                        in_=eB[:].rearrange("p (b s) -> p b s", b=GBL),
                        axis=AX.X,
                    )
                    rden = sb.tile([H, GBL], f32, tag=f"rden{g}")
                    nc.vector.reciprocal(out=rden[:], in_=den[:])
                    ctx2 = sb3.tile([H, GBL], f32, tag=f"ctx2{g}")
                    nc.vector.tensor_mul(out=ctx2[:], in0=ctxU[:], in1=rden[:])

                    # ---- decoder cells ----
                    mats0 = [(c_dwhh0T, h0d_ap[g]), (c_dwih0cT, ctx2[:])]
                    h0n = sb3.tile([H, GBL], f32, tag=f"h0d{g}")
                    c0d = lstm_cell(
                        f"d0{g}", dihv[:, t, g * 4 * GBL : (g + 1) * 4 * GBL],
                        mats0, c0d_ap[g], h0n[:],
                    )
                    h0d_ap[g] = h0n[:]
                    c0d_ap[g] = c0d[:]
                    mats1 = [(c_dwhh1T, h1d_ap[g]), (c_dwih1T, h0d_ap[g])]
                    h1n = sb3.tile([H, GBL], f32, tag=f"h1d{g}")
                    c1d = lstm_cell(f"d1{g}", c_db1f[:], mats1, c1d_ap[g], h1n[:])
                    h1d_ap[g] = h1n[:]
                    c1d_ap[g] = c1d[:]

                    # ---- gate (expert select, f32 margins) ----
                    psG = pscg[0:1, GBL * S : GBL * S + GBL]
                    nc.tensor.matmul(psG, lhsT=c_wd12[:, 0:1], rhs=h1d_ap[g],
                                     start=True, stop=False)
                    nc.tensor.matmul(psG, lhsT=c_wd12[:, 1:2], rhs=ctx2[:],
                                     start=False, stop=True)
                    sgn = sb.tile([1, GBL], f32, tag=f"sgn{g}")
                    nc.scalar.activation(out=sgn, in_=psG, func=AF.Sign,
                                         bias=c_gdb[0:1, 0:1])
                    nc.vector.tensor_scalar(
                        out=mS[0:1, moff + g * GBL : moff + (g + 1) * GBL],
                        in0=sgn[0:1, :], scalar1=1.0, scalar2=0.5,
                        op0=ALU.add, op1=ALU.mult,
                    )

                    # ---- payload (bf16) ----
                    payS = sb3.tile([H, 2 * GBL], bf16, tag=f"pay{g}")
                    nc.scalar.copy(out=payS[:, 0:GBL], in_=h1d_ap[g])
                    nc.scalar.copy(out=payS[:, GBL : 2 * GBL], in_=ctx2[:])
                    nc.sync.dma_start(
                        out=bounce[tc_off, 0 : 2 * H, gs].rearrange(
                            "(two p) b -> p two b", two=2
                        ),
                        in_=payS[:].rearrange("p (two b) -> p two b", two=2),
                    )
                if tc_off % 2 == 1:
                    nc.vector.tensor_copy(out=mCv[:], in_=mS[:])
                    nc.sync.dma_start(
                        out=bounce[tc_off - 1 : tc_off + 1, 2 * H, :],
                        in_=mCv[:].rearrange("p (s b) -> p s b", s=2),
                    )

                # ---- gather + pipelined MoE ----
                if tc_off == CHUNKS[ci] - 1 and DBG_PHASE >= 4:
                    gat = dr.tile([NCORES, CHUNKS[ci], PAYR, BL], bf16, tag=f"gat{CHUNKS[ci]}")
                    if DBG_COLLECTIVE:
                        nc.gpsimd.collective_compute(
                            "AllGather",
                            ALU.bypass,
                            replica_groups=[list(range(NCORES))],
                            ins=[bounce.opt()],
                            outs=[gat.opt()],
                        )
                    else:
                        for cc in range(NCORES):
                            nc.sync.dma_start(out=gat[cc], in_=bounce[:])
                    # flush any not-yet-emitted blocks of the previous gather
                    while gat_pend is not None and blk_next < blk_base + gat_pend[1] // 2:
                        moe_block(blk_next, gat_pend[0],
                                  2 * (blk_next - blk_base))
                        blk_next += 1
                    gat_pend = (gat, CHUNKS[ci])
                    blk_base = coff // 2
                    blk_next = blk_base
                    coff += CHUNKS[ci]
                    ci += 1
                elif (DBG_PHASE >= 4 and gat_pend is not None
                      and blk_next < blk_base + gat_pend[1] // 2):
                    moe_block(blk_next, gat_pend[0], 2 * (blk_next - blk_base))
                    blk_next += 1
            if run_dec and DBG_PHASE >= 4:
                while gat_pend is not None and blk_next < blk_base + gat_pend[1] // 2:
                    moe_block(blk_next, gat_pend[0], 2 * (blk_next - blk_base))
                    blk_next += 1

    nc.compile()
    return nc


def _prep_host(inputs):
    """Build the per-core input maps (pure layout/shard prep)."""
    f = np.float32

    def dblw(wT):
        # double the g-gate column block so one tanh(0.5*z) serves all gates
        wT = wT.copy()
        wT[:, 3 * H : 4 * H] *= 2.0
        return wT

    def dblb(bg):
        bg = bg.copy()
        bg[:, 3] *= 2.0
        return bg

    def ga(w):
        # [4H, D] pytorch gate order i,f,g,o -> i,f,o,g
        return np.concatenate([w[0:H], w[H : 2 * H], w[3 * H : 4 * H], w[2 * H : 3 * H]], axis=0)

    def gb(b):
        return np.concatenate([b[0:H], b[H : 2 * H], b[3 * H : 4 * H], b[2 * H : 3 * H]], axis=0)

    def bg_tile(b):
        return np.ascontiguousarray(gb(b).reshape(4, H).T).astype(f)

    def bfull(bg):
        # [H, 4] -> [H, 4*GBL] (each gate column replicated GBL times)
        return np.ascontiguousarray(np.repeat(bg, GBL, axis=1))

    emb = np.asarray(inputs["emb"], f)
    b1g = dblb(bg_tile(np.asarray(inputs["enc_b1"], f)))
    db1g = dblb(bg_tile(np.asarray(inputs["dec_b1"], f)))
    base = {
        "emb": np.ascontiguousarray(emb),
        "wih0T": dblw(np.ascontiguousarray(ga(np.asarray(inputs["enc_Wih0"], f)).T)),
        "whh0T": dblw(np.ascontiguousarray(ga(np.asarray(inputs["enc_Whh0"], f)).T) * 0.5),
        "b0g": dblb(bg_tile(np.asarray(inputs["enc_b0"], f))),
        "wih1T": dblw(np.ascontiguousarray(ga(np.asarray(inputs["enc_Wih1"], f)).T) * 0.5),
        "whh1T": dblw(np.ascontiguousarray(ga(np.asarray(inputs["enc_Whh1"], f)).T) * 0.5),
        "b1f": bfull(b1g),
        "dwhh0T": dblw(np.ascontiguousarray(ga(np.asarray(inputs["dec_Whh0"], f)).T) * 0.5),
        "db0g": dblb(bg_tile(np.asarray(inputs["dec_b0"], f))),
        "dwih1T": dblw(np.ascontiguousarray(ga(np.asarray(inputs["dec_Wih1"], f)).T) * 0.5),
        "dwhh1T": dblw(np.ascontiguousarray(ga(np.asarray(inputs["dec_Whh1"], f)).T) * 0.5),
        "db1f": bfull(db1g),
        "attWT": np.ascontiguousarray(np.asarray(inputs["att_W"], f).T) * 0.5,
        "attb": np.asarray(inputs["att_b"], f).reshape(H, 1),
        "attv": np.asarray(inputs["att_v"], f).reshape(H, 1).astype(ml_dtypes.bfloat16),
    }
    dwih0 = ga(np.asarray(inputs["dec_Wih0"], f))  # [512, E+H]
    dwih0T = np.ascontiguousarray(dwih0.T)         # [E+H, 512]
    base["dwih0xT"] = dblw(np.ascontiguousarray(dwih0T[0:E]))
    base["dwih0cT"] = dblw(np.ascontiguousarray(dwih0T[E : E + H]) * 0.5)

    gw = np.asarray(inputs["gate_W"], f)           # [2, 256]
    wd = (gw[0] - gw[1]) * 0.5
    base["wd12"] = np.ascontiguousarray(wd.reshape(2, H).T)
    gbv = np.asarray(inputs["gate_b"], f)
    base["gdb"] = np.array([[gbv[0] - gbv[1]]], f)

    expW = np.asarray(inputs["exp_W"], f)          # [2, V, 2H]
    expb = np.asarray(inputs["exp_b"], f)          # [2, V]
    src = np.asarray(inputs["src"], np.int32)
    trg = np.asarray(inputs["trg"], np.int32)

    in_maps = []
    for c in range(NCORES):
        m = dict(base)
        rows = slice(c * BL, (c + 1) * BL)
        m["src_idx"] = np.ascontiguousarray(src[rows].T).reshape(2, NTE // 2, 1)
        m["trg_idx"] = np.ascontiguousarray(trg[rows].T).reshape(2, NTD // 2, 1)
        vsl = slice(c * VS, (c + 1) * VS)
        W0 = expW[0, vsl]                          # [VS, 256]
        W1 = expW[1, vsl]
        w1T = W1.T * 0.5                           # [256, VS]
        wdT = (W0 - W1).T * 0.5
        m["w1a"] = np.ascontiguousarray(w1T[0:H]).astype(ml_dtypes.bfloat16)
        m["w1b"] = np.ascontiguousarray(w1T[H : 2 * H]).astype(ml_dtypes.bfloat16)
        m["wda"] = np.ascontiguousarray(wdT[0:H]).astype(ml_dtypes.bfloat16)
        m["wdb"] = np.ascontiguousarray(wdT[H : 2 * H]).astype(ml_dtypes.bfloat16)
        m["bias2"] = np.ascontiguousarray(
            np.stack([expb[0, vsl] - expb[1, vsl], expb[1, vsl]])
        ).astype(ml_dtypes.bfloat16)
        in_maps.append(m)
    return in_maps


last_results = None


def kernel(**inputs) -> np.ndarray:
    global last_results
    if "nc" not in _cache:
        _cache["nc"] = _build_program()
    nc = _cache["nc"]
    in_maps = _prep_host(inputs)
    trace = bool(os.environ.get("BASS_TRACE"))
    res = run_bass_kernel_spmd(
        nc, in_maps, core_ids=list(range(NCORES)), trace=trace
    )
    last_results = res
    # assemble: per-core out rows are (blk, c_src, s, b_local), cols = vocab shard
    parts = []
    for c in range(NCORES):
        o = res.results[c]["out"].reshape(NBLK, NCORES, 2, BL, VS)
        parts.append(np.transpose(o, (1, 3, 0, 2, 4)).reshape(B, T, VS))
    return np.concatenate(parts, axis=2).astype(np.float32)
